# revision 18
# baseline (speedup 1.0000x reference)
"""Trainium2 Bass kernel for AttentionalColorizedListenerDecoder.

Computes, for each example m:
    scores[m, p] = -(c_p - mu)^T Sigma (c_p - mu)   (p = 0..63, K = 128)
    out[m]      = softmax_p(scores[m])

Strategy (pure data-parallel over m across 8 cores, 512 examples/core):
  - Examples processed in octets (8) = 4 partition-packed pairs.
  - C loaded pair-packed: tile partitions = [ex_even p | ex_odd p], free = k.
  - PE transpose_mode turns C-pairs into C^T; a single DVE tensor_tensor add
    with a 0-step broadcast view of -mu^T produces s^T = (C - mu)^T in SBUF.
  - 8 col-tiled fp32 matmuls per octet: A = s Sigma, pair-packed in one PSUM
    bank (128 x 512).
  - s^T transposed back on PE -> s natural; ACT copies PSUM->SBUF.
  - One fused scalar_tensor_tensor (mult+mult+row-accumulate) per pair:
    scores_pos = sum_l A * s.
  - Per block: PE-transpose of the scores matrix, then a min-based softmax
    (softmax(-x) = exp(min - x)/sum) entirely on-chip.
"""

import numpy as np

M_TOTAL = 4096
P_DIM = 64
K_DIM = 128
N_CORES = 8
M_CORE = M_TOTAL // N_CORES  # 512
USE_F32R = False


def emit_body(tc, out_dram, c_dram, mu_dram, sig_dram, m_core):
    from concourse import masks, mybir

    nc = tc.nc
    f32 = mybir.dt.float32

    n_oct = m_core // 8
    blk = min(m_core, 256)          # examples per softmax block
    sig_per_dma = min(32, m_core)   # sigma examples per DMA (2 MiB)
    c_per_dma = min(64, m_core)     # C examples per DMA (2x 1 MiB)
    mu_per_dma = min(128, m_core)

    with (
        tc.tile_pool(name="const", bufs=1) as const_pool,
        tc.tile_pool(name="sig", bufs=3) as sig_pool,
        tc.tile_pool(name="cc", bufs=3) as c_pool,
        tc.tile_pool(name="mu", bufs=2) as mu_pool,
        tc.tile_pool(name="negmu", bufs=2) as negmu_pool,
        tc.tile_pool(name="sT", bufs=4) as sT_pool,
        tc.tile_pool(name="snat", bufs=4) as snat_pool,
        tc.tile_pool(name="scr", bufs=6) as scr_pool,
        tc.tile_pool(name="scores", bufs=2) as scores_pool,
        tc.tile_pool(name="soft", bufs=2) as soft_pool,
        tc.tile_pool(name="stat", bufs=2) as stat_pool,
        tc.tile_pool(name="psct", bufs=2, space="PSUM") as ps_ct_pool,
        tc.tile_pool(name="psA", bufs=4, space="PSUM") as ps_a_pool,
        tc.tile_pool(name="pssn", bufs=2, space="PSUM") as ps_sn_pool,
    ):
        ident = const_pool.tile([128, 128], f32)
        masks.make_identity(nc, ident[:])

        sig_tiles = {}
        c_tiles = {}
        negmu_tiles = {}
        scores_tile = [None]

        def load_sigma(s):  # sig_per_dma examples -> (128, n*128)
            t = sig_pool.tile([128, sig_per_dma * K_DIM], f32, tag="sig")
            src = sig_dram[s * sig_per_dma:(s + 1) * sig_per_dma]
            nc.sync.dma_start(
                t[:].rearrange("k (n l) -> k n l", n=sig_per_dma),
                src.rearrange("n k l -> k n l"),
            )
            sig_tiles[s] = t

        def load_c(a):  # c_per_dma examples pair-packed -> (128, n/2*128)
            npr = c_per_dma // 2
            t = c_pool.tile([128, npr * K_DIM], f32, tag="cc")
            src = c_dram[a * c_per_dma:(a + 1) * c_per_dma]
            split = src.rearrange("(t two) p k -> two p t k", two=2)
            nc.sync.dma_start(
                t[0:P_DIM, :].rearrange("p (t k) -> p t k", t=npr), split[0]
            )
            nc.sync.dma_start(
                t[P_DIM:128, :].rearrange("p (t k) -> p t k", t=npr), split[1]
            )
            c_tiles[a] = t

        def load_mu(cidx):  # mu_per_dma examples -> -mu^T (128 k, n ex)
            mt = mu_pool.tile([128, K_DIM], f32, tag="mu")
            nc.sync.dma_start(
                mt[:mu_per_dma, :K_DIM],
                mu_dram[cidx * mu_per_dma:(cidx + 1) * mu_per_dma],
            )
            ps = ps_a_pool.tile([128, 128], f32, tag="psA", name="ps_mu")
            nc.tensor.transpose(
                ps[:K_DIM, :mu_per_dma],
                mt[:mu_per_dma, :K_DIM],
                ident[:mu_per_dma, :mu_per_dma],
            )
            nt = negmu_pool.tile([128, mu_per_dma], f32, tag="negmu")
            nc.vector.tensor_scalar_mul(nt[:K_DIM, :mu_per_dma], ps[:K_DIM, :mu_per_dma], -1.0)
            negmu_tiles[cidx] = nt

        def softmax_and_store(b):
            # scores_tile: (128, blk//2) columns = pairs of block b
            sc = scores_tile[0]
            npair = blk // 2
            ps = ps_a_pool.tile([128, 128], f32, tag="psA", name="ps_sc")
            nc.tensor.transpose(ps[:npair, :128], sc[:, :npair], ident[:])
            # row r = pair r: cols 0:64 = ex 2r scores, 64:128 = ex 2r+1
            m0 = stat_pool.tile([128, 1], f32, tag="m0")
            m1 = stat_pool.tile([128, 1], f32, tag="m1")
            nc.vector.tensor_reduce(m0[:npair], ps[:npair, 0:P_DIM], axis=mybir.AxisListType.X, op=mybir.AluOpType.min)
            nc.vector.tensor_reduce(m1[:npair], ps[:npair, P_DIM:128], axis=mybir.AxisListType.X, op=mybir.AluOpType.min)
            eb = soft_pool.tile([128, 128], f32, tag="eb")
            nc.scalar.activation(eb[:npair, 0:P_DIM], ps[:npair, 0:P_DIM], mybir.ActivationFunctionType.Exp, bias=m0[:npair], scale=-1.0)
            nc.scalar.activation(eb[:npair, P_DIM:128], ps[:npair, P_DIM:128], mybir.ActivationFunctionType.Exp, bias=m1[:npair], scale=-1.0)
            sums = stat_pool.tile([128, 2], f32, tag="sums")
            nc.vector.tensor_reduce(sums[:npair], eb[:npair].rearrange("r (two p) -> r two p", two=2), axis=mybir.AxisListType.X, op=mybir.AluOpType.add)
            rec = stat_pool.tile([128, 2], f32, tag="rec")
            nc.vector.reciprocal(rec[:npair], sums[:npair])
            ob = soft_pool.tile([128, 128], f32, tag="ob")
            nc.vector.tensor_scalar_mul(ob[:npair, 0:P_DIM], eb[:npair, 0:P_DIM], rec[:npair, 0:1])
            nc.vector.tensor_scalar_mul(ob[:npair, P_DIM:128], eb[:npair, P_DIM:128], rec[:npair, 1:2])
            dst = out_dram[b * blk:(b + 1) * blk]
            nc.sync.dma_start(dst.rearrange("(r two) p -> r (two p)", two=2), ob[:npair, :])

        for j in range(n_oct):
            if j % (max(sig_per_dma // 8, 1)) == 0:
                load_sigma(j // max(sig_per_dma // 8, 1))
            if j % (max(c_per_dma // 8, 1)) == 0:
                load_c(j // max(c_per_dma // 8, 1))
            if j % (max(mu_per_dma // 8, 1)) == 0:
                load_mu(j // max(mu_per_dma // 8, 1))
            if j % (blk // 8) == 0:
                scores_tile[0] = scores_pool.tile([128, blk // 2], f32, tag="scores", name="scores")

            sig_t = sig_tiles[j // max(sig_per_dma // 8, 1)]
            sig_off = (j % max(sig_per_dma // 8, 1)) * 8 * K_DIM
            c_t = c_tiles[j // max(c_per_dma // 8, 1)]
            c_off = (j % max(c_per_dma // 8, 1)) * 4 * K_DIM
            nm_t = negmu_tiles[j // max(mu_per_dma // 8, 1)]
            nm_off = (j * 8) % mu_per_dma

            # 1) C^T for the 4 pairs of this octet
            ps_ct = ps_ct_pool.tile([128, 512], f32, tag="psct")
            for t in range(4):
                nc.tensor.transpose(
                    ps_ct[:, 128 * t:128 * (t + 1)],
                    c_t[:, c_off + 128 * t: c_off + 128 * (t + 1)],
                    ident[:],
                )

            # 2) s^T = C^T - mu (one DVE op, 0-step broadcast of -mu^T)
            sT = sT_pool.tile([128, 512], f32, tag="sT")
            nm_view = nm_t[:, nm_off:nm_off + 8].unsqueeze(2).broadcast_to((128, 8, P_DIM))
            nc.vector.tensor_add(
                sT[:].rearrange("k (e p) -> k e p", e=8),
                ps_ct[:].rearrange("k (e p) -> k e p", e=8),
                nm_view,
            )

            # 3) A = s Sigma, pair-packed: 8 col-tiled matmuls
            ps_a = ps_a_pool.tile([128, 512], f32, tag="psA")
            for b in range(8):
                t, h = b // 2, b % 2
                lhs = sT[:, 64 * b:64 * (b + 1)]
                rhs = sig_t[:, sig_off + 128 * b: sig_off + 128 * (b + 1)]
                if USE_F32R:
                    lhs = lhs.bitcast(mybir.dt.float32r)
                    rhs = rhs.bitcast(mybir.dt.float32r)
                nc.tensor.matmul(
                    ps_a[64 * h:64 * (h + 1), 128 * t:128 * (t + 1)],
                    lhs,
                    rhs,
                    start=True,
                    stop=True,
                    tile_position=(0, 64 * h),
                )

            # 4) s natural: transpose s^T pairs back, copy PSUM->SBUF
            ps_sn = ps_sn_pool.tile([128, 512], f32, tag="pssn")
            for t in range(4):
                nc.tensor.transpose(
                    ps_sn[:, 128 * t:128 * (t + 1)],
                    sT[:, 128 * t:128 * (t + 1)],
                    ident[:],
                )
            snat = snat_pool.tile([128, 512], f32, tag="snat")
            nc.scalar.activation(snat[:], ps_sn[:], mybir.ActivationFunctionType.Identity)

            # 5) scores_pos[pair] = rowsum(A * s), fused mult+accumulate on DVE
            sc = scores_tile[0]
            col0 = (j % (blk // 8)) * 4
            scr = scr_pool.tile([128, 512], f32, tag="scr")
            for t in range(4):
                nc.vector.scalar_tensor_tensor(
                    out=scr[:, 128 * t:128 * (t + 1)],
                    in0=ps_a[:, 128 * t:128 * (t + 1)],
                    scalar=1.0,
                    in1=snat[:, 128 * t:128 * (t + 1)],
                    op0=mybir.AluOpType.mult,
                    op1=mybir.AluOpType.mult,
                    accum_out=sc[:, col0 + t:col0 + t + 1],
                )

            if (j + 1) % (blk // 8) == 0:
                softmax_and_store(j // (blk // 8))


def build_nc(m_core: int = M_CORE, repeat: int = 1):
    import concourse.tile as tile
    from concourse import bacc, mybir

    f32 = mybir.dt.float32
    nc = bacc.Bacc("TRN2", target_bir_lowering=False, debug=False)
    c_dram = nc.dram_tensor("color_seqs", [m_core, P_DIM, K_DIM], f32, kind="ExternalInput").ap()
    mu_dram = nc.dram_tensor("mew", [m_core, K_DIM], f32, kind="ExternalInput").ap()
    sig_dram = nc.dram_tensor("sigma", [m_core, K_DIM, K_DIM], f32, kind="ExternalInput").ap()
    out_dram = nc.dram_tensor("out", [m_core, P_DIM], f32, kind="ExternalOutput").ap()

    with tile.TileContext(nc) as tc:
        if repeat > 1:
            with tc.For_i(0, repeat, 1):
                emit_body(tc, out_dram, c_dram, mu_dram, sig_dram, m_core)
        else:
            emit_body(tc, out_dram, c_dram, mu_dram, sig_dram, m_core)

    nc.finalize()
    return nc


_NC = {}


def _get_nc(m_core: int):
    if m_core not in _NC:
        _NC[m_core] = build_nc(m_core)
    return _NC[m_core]


def kernel(color_seqs, mew, sigma):
    from concourse.bass_utils import run_bass_kernel_spmd

    color_seqs = np.asarray(color_seqs, dtype=np.float32)
    mew = np.asarray(mew, dtype=np.float32)
    sigma = np.asarray(sigma, dtype=np.float32)
    assert color_seqs.shape == (M_TOTAL, P_DIM, K_DIM)

    nc = _get_nc(M_CORE)
    in_maps = [
        {
            "color_seqs": color_seqs[i * M_CORE:(i + 1) * M_CORE],
            "mew": mew[i * M_CORE:(i + 1) * M_CORE],
            "sigma": sigma[i * M_CORE:(i + 1) * M_CORE],
        }
        for i in range(N_CORES)
    ]
    res = run_bass_kernel_spmd(nc, in_maps, core_ids=list(range(N_CORES)))
    return np.concatenate([res.results[i]["out"] for i in range(N_CORES)], axis=0)


# revision 21
# speedup vs baseline: 6.5000x; 6.5000x over previous
"""Trainium2 Bass kernel for AttentionalColorizedListenerDecoder.

Computes, for each example m:
    scores[m, p] = -(c_p - mu)^T Sigma (c_p - mu)   (p = 0..63, K = 128)
    out[m]      = softmax_p(scores[m])

Strategy (pure data-parallel over m across 8 cores, 512 examples/core):
  - Examples processed in octets (8) = 4 partition-packed pairs.
  - C loaded pair-packed: tile partitions = [ex_even p | ex_odd p], free = k.
  - PE transpose_mode turns C-pairs into C^T; a single DVE tensor_tensor add
    with a 0-step broadcast view of -mu^T produces s^T = (C - mu)^T in SBUF.
  - 8 col-tiled fp32 matmuls per octet: A = s Sigma, pair-packed in one PSUM
    bank (128 x 512).
  - s^T transposed back on PE -> s natural; ACT copies PSUM->SBUF.
  - One fused scalar_tensor_tensor (mult+mult+row-accumulate) per pair:
    scores_pos = sum_l A * s.
  - Per block: PE-transpose of the scores matrix, then a min-based softmax
    (softmax(-x) = exp(min - x)/sum) entirely on-chip.
"""

import numpy as np

M_TOTAL = 4096
P_DIM = 64
K_DIM = 128
N_CORES = 8
M_CORE = M_TOTAL // N_CORES  # 512
USE_F32R = False
DOUBLE_MM = False


def emit_body(tc, out_dram, c_dram, mu_dram, sig_dram, m_core):
    from concourse import masks, mybir

    nc = tc.nc
    f32 = mybir.dt.float32

    n_oct = m_core // 8
    blk = min(m_core, 256)          # examples per softmax block
    sig_per_dma = min(32, m_core)   # sigma examples per DMA (2 MiB)
    c_per_dma = min(64, m_core)     # C examples per DMA (2x 1 MiB)
    mu_per_dma = min(128, m_core)

    with (
        tc.tile_pool(name="const", bufs=1) as const_pool,
        tc.tile_pool(name="sig", bufs=3) as sig_pool,
        tc.tile_pool(name="cc", bufs=3) as c_pool,
        tc.tile_pool(name="mu", bufs=2) as mu_pool,
        tc.tile_pool(name="negmu", bufs=2) as negmu_pool,
        tc.tile_pool(name="sT", bufs=4) as sT_pool,
        tc.tile_pool(name="snat", bufs=4) as snat_pool,
        tc.tile_pool(name="scr", bufs=6) as scr_pool,
        tc.tile_pool(name="scores", bufs=2) as scores_pool,
        tc.tile_pool(name="soft", bufs=2) as soft_pool,
        tc.tile_pool(name="stat", bufs=2) as stat_pool,
        tc.tile_pool(name="psct", bufs=3, space="PSUM") as ps_ct_pool,
        tc.tile_pool(name="psA", bufs=3, space="PSUM") as ps_a_pool,
        tc.tile_pool(name="pssn", bufs=2, space="PSUM") as ps_sn_pool,
    ):
        ident = const_pool.tile([128, 128], f32)
        masks.make_identity(nc, ident[:])

        sig_tiles = {}
        c_tiles = {}
        negmu_tiles = {}
        scores_tile = [None]

        def load_sigma(s):  # sig_per_dma examples -> (128, n*128)
            t = sig_pool.tile([128, sig_per_dma * K_DIM], f32, tag="sig")
            src = sig_dram[s * sig_per_dma:(s + 1) * sig_per_dma]
            nc.sync.dma_start(
                t[:].rearrange("k (n l) -> k n l", n=sig_per_dma),
                src.rearrange("n k l -> k n l"),
            )
            sig_tiles[s] = t

        def load_c(a):  # c_per_dma examples pair-packed -> (128, n/2*128)
            npr = c_per_dma // 2
            t = c_pool.tile([128, npr * K_DIM], f32, tag="cc")
            src = c_dram[a * c_per_dma:(a + 1) * c_per_dma]
            split = src.rearrange("(t two) p k -> two p t k", two=2)
            nc.sync.dma_start(
                t[0:P_DIM, :].rearrange("p (t k) -> p t k", t=npr), split[0]
            )
            nc.sync.dma_start(
                t[P_DIM:128, :].rearrange("p (t k) -> p t k", t=npr), split[1]
            )
            c_tiles[a] = t

        def load_mu(cidx):  # mu_per_dma examples -> -mu^T (128 k, n ex)
            mt = mu_pool.tile([128, K_DIM], f32, tag="mu")
            nc.sync.dma_start(
                mt[:mu_per_dma, :K_DIM],
                mu_dram[cidx * mu_per_dma:(cidx + 1) * mu_per_dma],
            )
            ps = ps_a_pool.tile([128, 128], f32, tag="psA", name="ps_mu")
            nc.tensor.transpose(
                ps[:K_DIM, :mu_per_dma],
                mt[:mu_per_dma, :K_DIM],
                ident[:mu_per_dma, :mu_per_dma],
            )
            nt = negmu_pool.tile([128, mu_per_dma], f32, tag="negmu")
            nc.vector.tensor_scalar_mul(nt[:K_DIM, :mu_per_dma], ps[:K_DIM, :mu_per_dma], -1.0)
            negmu_tiles[cidx] = nt

        def softmax_and_store(b):
            # scores_tile: (128, blk//2) columns = pairs of block b
            sc = scores_tile[0]
            npair = blk // 2
            ps = ps_a_pool.tile([128, 128], f32, tag="psA", name="ps_sc")
            nc.tensor.transpose(ps[:npair, :128], sc[:, :npair], ident[:])
            # row r = pair r: cols 0:64 = ex 2r scores, 64:128 = ex 2r+1
            m0 = stat_pool.tile([128, 1], f32, tag="m0")
            m1 = stat_pool.tile([128, 1], f32, tag="m1")
            nc.vector.tensor_reduce(m0[:npair], ps[:npair, 0:P_DIM], axis=mybir.AxisListType.X, op=mybir.AluOpType.min)
            nc.vector.tensor_reduce(m1[:npair], ps[:npair, P_DIM:128], axis=mybir.AxisListType.X, op=mybir.AluOpType.min)
            eb = soft_pool.tile([128, 128], f32, tag="eb")
            nc.scalar.activation(eb[:npair, 0:P_DIM], ps[:npair, 0:P_DIM], mybir.ActivationFunctionType.Exp, bias=m0[:npair], scale=-1.0)
            nc.scalar.activation(eb[:npair, P_DIM:128], ps[:npair, P_DIM:128], mybir.ActivationFunctionType.Exp, bias=m1[:npair], scale=-1.0)
            sums = stat_pool.tile([128, 2], f32, tag="sums")
            nc.vector.tensor_reduce(sums[:npair], eb[:npair].rearrange("r (two p) -> r two p", two=2), axis=mybir.AxisListType.X, op=mybir.AluOpType.add)
            rec = stat_pool.tile([128, 2], f32, tag="rec")
            nc.vector.reciprocal(rec[:npair], sums[:npair])
            ob = soft_pool.tile([128, 128], f32, tag="ob")
            nc.vector.tensor_scalar_mul(ob[:npair, 0:P_DIM], eb[:npair, 0:P_DIM], rec[:npair, 0:1])
            nc.vector.tensor_scalar_mul(ob[:npair, P_DIM:128], eb[:npair, P_DIM:128], rec[:npair, 1:2])
            dst = out_dram[b * blk:(b + 1) * blk]
            nc.sync.dma_start(dst.rearrange("(r two) p -> r (two p)", two=2), ob[:npair, :])

        GRP = 2 if n_oct % 2 == 0 else 1

        def ensure_loads(j):
            if j % (max(sig_per_dma // 8, 1)) == 0:
                load_sigma(j // max(sig_per_dma // 8, 1))
            if j % (max(c_per_dma // 8, 1)) == 0:
                load_c(j // max(c_per_dma // 8, 1))
            if j % (max(mu_per_dma // 8, 1)) == 0:
                load_mu(j // max(mu_per_dma // 8, 1))
            if j % (blk // 8) == 0:
                scores_tile[0] = scores_pool.tile([128, blk // 2], f32, tag="scores", name="scores")

        def oct_refs(j):
            sig_t = sig_tiles[j // max(sig_per_dma // 8, 1)]
            sig_off = (j % max(sig_per_dma // 8, 1)) * 8 * K_DIM
            c_t = c_tiles[j // max(c_per_dma // 8, 1)]
            c_off = (j % max(c_per_dma // 8, 1)) * 4 * K_DIM
            nm_t = negmu_tiles[j // max(mu_per_dma // 8, 1)]
            nm_off = (j * 8) % mu_per_dma
            return sig_t, sig_off, c_t, c_off, nm_t, nm_off

        for g in range(n_oct // GRP):
            octs = [g * GRP + i for i in range(GRP)]
            for j in octs:
                ensure_loads(j)
            refs = {j: oct_refs(j) for j in octs}

            # Phase 1: C^T transposes for all octets of the group
            cts = {}
            for j in octs:
                _, _, c_t, c_off, _, _ = refs[j]
                ps_ct = ps_ct_pool.tile([128, 512], f32, tag="psct", name="ps_ct")
                for t in range(4):
                    nc.tensor.transpose(
                        ps_ct[:, 128 * t:128 * (t + 1)],
                        c_t[:, c_off + 128 * t: c_off + 128 * (t + 1)],
                        ident[:],
                    )
                cts[j] = ps_ct

            # Phase 2: s^T = C^T - mu (one DVE op per octet)
            sTs = {}
            for j in octs:
                _, _, _, _, nm_t, nm_off = refs[j]
                sT = sT_pool.tile([128, 512], f32, tag="sT", name="sT")
                nm_view = nm_t[:, nm_off:nm_off + 8].unsqueeze(2).broadcast_to((128, 8, P_DIM))
                nc.vector.tensor_add(
                    sT[:].rearrange("k (e p) -> k e p", e=8),
                    cts[j][:].rearrange("k (e p) -> k e p", e=8),
                    nm_view,
                )
                sTs[j] = sT

            # Phase 3: all matmuls of the group back-to-back (dense PE burst)
            ps_as = {}
            for j in octs:
                sig_t, sig_off, _, _, _, _ = refs[j]
                sT = sTs[j]
                ps_a = ps_a_pool.tile([128, 512], f32, tag="psA", name="ps_a")
                for b in range(8):
                    t, h = b // 2, b % 2
                    nc.tensor.matmul(
                        ps_a[64 * h:64 * (h + 1), 128 * t:128 * (t + 1)],
                        sT[:, 64 * b:64 * (b + 1)],
                        sig_t[:, sig_off + 128 * b: sig_off + 128 * (b + 1)],
                        start=True,
                        stop=True,
                        tile_position=(0, 64 * h),
                    )
                ps_as[j] = ps_a

            # Phase 4: s natural via back-transposes, ACT copy PSUM->SBUF
            snats = {}
            for j in octs:
                sT = sTs[j]
                ps_sn = ps_sn_pool.tile([128, 512], f32, tag="pssn", name="ps_sn")
                for t in range(4):
                    nc.tensor.transpose(
                        ps_sn[:, 128 * t:128 * (t + 1)],
                        sT[:, 128 * t:128 * (t + 1)],
                        ident[:],
                    )
                snat = snat_pool.tile([128, 512], f32, tag="snat", name="snat")
                nc.scalar.activation(snat[:], ps_sn[:], mybir.ActivationFunctionType.Identity)
                snats[j] = snat

            # Phase 5: fused rowdot per pair
            for j in octs:
                sc = scores_tile[0]
                col0 = (j % (blk // 8)) * 4
                scr = scr_pool.tile([128, 512], f32, tag="scr", name="scr")
                for t in range(4):
                    nc.vector.scalar_tensor_tensor(
                        out=scr[:, 128 * t:128 * (t + 1)],
                        in0=ps_as[j][:, 128 * t:128 * (t + 1)],
                        scalar=1.0,
                        in1=snats[j][:, 128 * t:128 * (t + 1)],
                        op0=mybir.AluOpType.mult,
                        op1=mybir.AluOpType.mult,
                        accum_out=sc[:, col0 + t:col0 + t + 1],
                    )

            j = octs[-1]
            if (j + 1) % (blk // 8) == 0:
                softmax_and_store(j // (blk // 8))


def build_nc(m_core: int = M_CORE, repeat: int = 1):
    import concourse.tile as tile
    from concourse import bacc, mybir

    f32 = mybir.dt.float32
    nc = bacc.Bacc("TRN2", target_bir_lowering=False, debug=False)
    c_dram = nc.dram_tensor("color_seqs", [m_core, P_DIM, K_DIM], f32, kind="ExternalInput").ap()
    mu_dram = nc.dram_tensor("mew", [m_core, K_DIM], f32, kind="ExternalInput").ap()
    sig_dram = nc.dram_tensor("sigma", [m_core, K_DIM, K_DIM], f32, kind="ExternalInput").ap()
    out_dram = nc.dram_tensor("out", [m_core, P_DIM], f32, kind="ExternalOutput").ap()

    with tile.TileContext(nc) as tc:
        if repeat > 1:
            with tc.For_i(0, repeat, 1):
                emit_body(tc, out_dram, c_dram, mu_dram, sig_dram, m_core)
        else:
            emit_body(tc, out_dram, c_dram, mu_dram, sig_dram, m_core)

    nc.finalize()
    return nc


_NC = {}


def _get_nc(m_core: int):
    if m_core not in _NC:
        _NC[m_core] = build_nc(m_core)
    return _NC[m_core]


def kernel(color_seqs, mew, sigma):
    from concourse.bass_utils import run_bass_kernel_spmd

    color_seqs = np.asarray(color_seqs, dtype=np.float32)
    mew = np.asarray(mew, dtype=np.float32)
    sigma = np.asarray(sigma, dtype=np.float32)
    assert color_seqs.shape == (M_TOTAL, P_DIM, K_DIM)

    nc = _get_nc(M_CORE)
    in_maps = [
        {
            "color_seqs": color_seqs[i * M_CORE:(i + 1) * M_CORE],
            "mew": mew[i * M_CORE:(i + 1) * M_CORE],
            "sigma": sigma[i * M_CORE:(i + 1) * M_CORE],
        }
        for i in range(N_CORES)
    ]
    res = run_bass_kernel_spmd(nc, in_maps, core_ids=list(range(N_CORES)))
    return np.concatenate([res.results[i]["out"] for i in range(N_CORES)], axis=0)
